# revision 10
# baseline (speedup 1.0000x reference)
"""GCCF (3-layer bipartite GNN) Trainium2 kernel, 8-core SPMD.

Strategy: 1D row partition of users (2500/core) and movies (1250/core).
Adjacency shards are pre-transposed on host so the tensor engine contracts
over the partition dimension with the (replicated) activation table as the
stationary operand.  Each layer:
    aggT = (adj_shard @ act_full)^T          accumulated E-major in PSUM
    X    = aggT + selfT                      (DVE, PSUM->SBUF)
    newT = LeakyRelu(W @ X + 2b)             (PE + ScalarE)
    new  = transpose(newT)                   (PE transpose, 128-col tiles)
    AllGather(new) across the 8 cores -> full activation for next layer
Scoring: per-core batch shard of 1024 interactions, dma_gather of rows from
every depth's user table and (out_W-weighted) movie table, fused
multiply+reduce on DVE.
"""

import sys

sys.path.insert(0, "/opt/trn_rl_repo")

import numpy as np
import ml_dtypes

import concourse.bass as bass
import concourse.mybir as mybir
import concourse.tile as tile
from concourse import bacc
from concourse.bass_utils import run_bass_kernel_spmd
from concourse.masks import make_identity

F32 = mybir.dt.float32
BF16 = mybir.dt.bfloat16
I16 = mybir.dt.int16

NCORES = 8
NU, NM, E, L, B = 20000, 10000, 64, 3, 8192
SLOPE = 0.01
ADJ_DT = F32  # adjacency / table dtype for the big matmuls

PSUM_CHUNK = 512  # fp32 PSUM bank


def _cdiv(a, b):
    return (a + b - 1) // b


def build_kernel(nu, nm, e, n_layers, bs, ncores, adj_dt, do_score=True,
                 score_inputs_only=False):
    """Build the SPMD bass program (identical on every core)."""
    us, ms = nu // ncores, nm // ncores
    esz = mybir.dt.size(adj_dt)
    cast = adj_dt != F32
    rg = [list(range(ncores))]
    LRELU = mybir.ActivationFunctionType.Lrelu

    nc = bacc.Bacc(None, target_bir_lowering=False, num_devices=ncores)

    # ---- parameters (per-core values supplied from host) ----
    uat = nc.declare_dram_parameter("uat", [nm, us], adj_dt, isOutput=False)
    mat = nc.declare_dram_parameter("mat", [nu, ms], adj_dt, isOutput=False)
    u0 = nc.declare_dram_parameter("u0", [nu, e], F32, isOutput=False)
    m0 = nc.declare_dram_parameter("m0", [nm, e], F32, isOutput=False)
    m0w = nc.declare_dram_parameter("m0w", [nm, e], F32, isOutput=False)
    u0T = nc.declare_dram_parameter("u0T", [e, us], F32, isOutput=False)
    m0T = nc.declare_dram_parameter("m0T", [e, ms], F32, isOutput=False)
    wuT = nc.declare_dram_parameter("wuT", [n_layers, e, e], F32, isOutput=False)
    wmT = nc.declare_dram_parameter("wmT", [n_layers, e, e], F32, isOutput=False)
    buT = nc.declare_dram_parameter("buT", [e, n_layers], F32, isOutput=False)
    bmT = nc.declare_dram_parameter("bmT", [e, n_layers], F32, isOutput=False)
    owT = nc.declare_dram_parameter("owT", [e, n_layers + 1], F32, isOutput=False)
    ob = nc.declare_dram_parameter("ob", [128, 1], F32, isOutput=False)
    nidx = _cdiv(bs, 16)
    uidx = nc.declare_dram_parameter("uidx", [128, nidx], I16, isOutput=False)
    midx = nc.declare_dram_parameter("midx", [128, nidx], I16, isOutput=False)

    bchunks = _cdiv(bs, 128)
    scores_o = nc.declare_dram_parameter("scores", [128, bchunks], F32, isOutput=True)
    u3_o = nc.declare_dram_parameter("u3", [us, e], F32, isOutput=True)
    m3_o = nc.declare_dram_parameter("m3", [ms, e], F32, isOutput=True)

    # ---- internal DRAM (AllGather bounce + outputs) ----
    u_sh, U_full, m_sh, M_full, mw_sh, MW_full = {}, {}, {}, {}, {}, {}
    for ll in range(1, n_layers + 1):
        u_sh[ll] = nc.dram_tensor(f"u_sh{ll}", [us, e], F32)
        U_full[ll] = nc.dram_tensor(f"U_full{ll}", [nu, e], F32, addr_space="Shared")
        mw_sh[ll] = nc.dram_tensor(f"mw_sh{ll}", [ms, e], F32)
        MW_full[ll] = nc.dram_tensor(f"MW_full{ll}", [nm, e], F32, addr_space="Shared")
        if ll < n_layers:
            m_sh[ll] = nc.dram_tensor(f"m_sh{ll}", [ms, e], F32)
            M_full[ll] = nc.dram_tensor(f"M_full{ll}", [nm, e], F32, addr_space="Shared")

    ukt = _cdiv(nu, 128)  # user row tiles (table for movie side)
    mkt = _cdiv(nm, 128)  # movie row tiles (table for user side)

    # DMA group sizing for adjacency slabs: ~2.5 MiB per dma_start
    def tiles_per_group(ncols):
        per_tile = 128 * ncols * esz
        return max(1, min(8, round(2.5e6 / per_tile)))

    with tile.TileContext(nc) as tc:
        with (
            tc.tile_pool(name="const", bufs=1) as constp,
            tc.tile_pool(name="tab", bufs=2) as tabp,
            tc.tile_pool(name="utab", bufs=1) as utabp,
            tc.tile_pool(name="slab", bufs=2) as slabp,
            tc.tile_pool(name="act", bufs=2) as actp,
            tc.tile_pool(name="work", bufs=2) as workp,
            tc.tile_pool(name="nat", bufs=1) as natp,
            tc.tile_pool(name="psacc", bufs=1, space="PSUM") as psacc,
            tc.tile_pool(name="psw", bufs=2, space="PSUM") as psw,
            tc.tile_pool(name="pstr", bufs=1, space="PSUM") as pstr,
        ):
            # ---- constants ----
            ident = constp.tile([e, e], F32)
            make_identity(nc, ident[:])
            wu_t = constp.tile([e, n_layers, e], F32)
            nc.sync.dma_start(wu_t[:], wuT[:].rearrange("l k m -> k l m"))
            wm_t = constp.tile([e, n_layers, e], F32)
            nc.sync.dma_start(wm_t[:], wmT[:].rearrange("l k m -> k l m"))
            bu_t = constp.tile([e, n_layers], F32)
            nc.sync.dma_start(bu_t[:], buT[:])
            bm_t = constp.tile([e, n_layers], F32)
            nc.sync.dma_start(bm_t[:], bmT[:])
            ow_t = constp.tile([e, n_layers + 1], F32)
            nc.sync.dma_start(ow_t[:], owT[:])
            ob_t = constp.tile([128, 1], F32)
            nc.sync.dma_start(ob_t[:], ob[:])
            uidx_t = constp.tile([128, nidx], I16)
            nc.sync.dma_start(uidx_t[:], uidx[:])
            midx_t = constp.tile([128, nidx], I16)
            nc.sync.dma_start(midx_t[:], midx[:])

            def load_table(pool, tag, src_h, rows):
                """[rows, e] f32 DRAM -> [128, ktiles, e] adj_dt SBUF table."""
                kt = _cdiv(rows, 128)
                t = pool.tile([128, kt, e], adj_dt, tag=tag)
                eng = nc.gpsimd if cast else nc.sync
                ft, tw = divmod(rows, 128)
                if ft:
                    eng.dma_start(
                        t[:, 0:ft, :],
                        src_h[0 : ft * 128, :].rearrange("(t p) e -> p t e", p=128),
                    )
                if tw:
                    eng.dma_start(t[0:tw, ft, :], src_h[ft * 128 : rows, :])
                return t

            def agg_side(adj_h, rows, ncols, table, selfT, w_sl, b_sl, outT):
                """outT[e, ncols] = LRelu(W @ ((adjT @ table-act)^T + selfT) + b).

                adj_h: [rows, ncols] transposed-adjacency DRAM (contraction
                rows on axis 0); table: [128, kt, e] stationary activations.
                """
                kt_total = _cdiv(rows, 128)
                nch = _cdiv(ncols, PSUM_CHUNK)
                acc = psacc.tile([e, nch, PSUM_CHUNK], F32, tag="acc")
                tpg = tiles_per_group(ncols)
                ngroups = _cdiv(kt_total, tpg)
                for g in range(ngroups):
                    t0 = g * tpg
                    t1 = min(t0 + tpg, kt_total)
                    r0 = t0 * 128
                    r1 = min(t1 * 128, rows)
                    full = (r1 - r0) // 128  # full 128-row tiles in group
                    tw = (r1 - r0) - full * 128
                    slab = slabp.tile([128, tpg, ncols], adj_dt, tag="slab")
                    if full:
                        nc.sync.dma_start(
                            slab[:, 0:full, :],
                            adj_h[r0 : r0 + full * 128, :].rearrange(
                                "(t p) n -> p t n", p=128
                            ),
                        )
                    if tw:
                        nc.sync.dma_start(
                            slab[0:tw, full, :], adj_h[r0 + full * 128 : r1, :]
                        )
                    for i in range(t1 - t0):
                        kt = t0 + i
                        pw = 128 if (kt + 1) * 128 <= rows else rows - kt * 128
                        for j in range(nch):
                            c0 = j * PSUM_CHUNK
                            cw = min(PSUM_CHUNK, ncols - c0)
                            nc.tensor.matmul(
                                acc[:, j, 0:cw],
                                table[0:pw, kt, :],
                                slab[0:pw, i, c0 : c0 + cw],
                                start=(kt == 0),
                                stop=(kt == kt_total - 1),
                            )
                # X = acc + selfT ; W apply ; LRelu
                for j in range(nch):
                    c0 = j * PSUM_CHUNK
                    cw = min(PSUM_CHUNK, ncols - c0)
                    X = workp.tile([e, PSUM_CHUNK], F32, tag="x")
                    nc.vector.tensor_add(
                        X[:, 0:cw], acc[:, j, 0:cw], selfT[:, c0 : c0 + cw]
                    )
                    pw2 = psw.tile([e, PSUM_CHUNK], F32, tag="pw")
                    nc.tensor.matmul(
                        pw2[:, 0:cw], w_sl, X[:, 0:cw], start=True, stop=True
                    )
                    nc.scalar.activation(
                        outT[:, c0 : c0 + cw],
                        pw2[:, 0:cw],
                        LRELU,
                        bias=b_sl,
                        scale=1.0,
                        alpha=SLOPE,
                    )

            def to_nat(srcT, ncols, tag):
                """[e, ncols] SBUF -> [128, ntil, e] SBUF row-major tiles."""
                ntil = _cdiv(ncols, 128)
                nat = natp.tile([128, ntil, e], F32, tag=tag)
                for t in range(ntil):
                    c0 = t * 128
                    w = min(128, ncols - c0)
                    tp = pstr.tile([128, e], F32, tag="ptr")
                    nc.tensor.transpose(tp[0:w, :], srcT[:, c0 : c0 + w], ident[:])
                    nc.vector.tensor_copy(nat[0:w, t, :], tp[0:w, :])
                return nat

            def nat_to_dram(nat, ncols, dst_h):
                ft, tw = divmod(ncols, 128)
                if ft:
                    nc.sync.dma_start(
                        dst_h[0 : ft * 128, :].rearrange("(t p) e -> p t e", p=128),
                        nat[:, 0:ft, :],
                    )
                if tw:
                    nc.sync.dma_start(dst_h[ft * 128 : ncols, :], nat[0:tw, ft, :])

            # ---- initial self terms (E-major shards) ----
            uT = actp.tile([e, us], F32, tag="ut")
            nc.sync.dma_start(uT[:], u0T[:])
            mT = actp.tile([e, ms], F32, tag="mt")
            nc.sync.dma_start(mT[:], m0T[:])

            for ll in range(1, n_layers + 1):
                li = ll - 1
                # --- user side: u_new = f(user_adj @ m_prev + u_prev) ---
                mtab = load_table(
                    tabp, "mtab", m0 if ll == 1 else M_full[ll - 1], nm
                )
                u_newT = actp.tile([e, us], F32, tag="ut")
                agg_side(
                    uat, nm, us, mtab, uT,
                    wu_t[:, li, :], bu_t[:, li : li + 1], u_newT,
                )
                unat = to_nat(u_newT, us, "unat")
                nat_to_dram(unat, us, u_sh[ll])
                if ll == n_layers:
                    nat_to_dram(unat, us, u3_o)
                nc.gpsimd.collective_compute(
                    "AllGather", mybir.AluOpType.bypass, replica_groups=rg,
                    ins=[u_sh[ll][:]], outs=[U_full[ll][:]],
                )

                # --- movie side: m_new = f(movie_adj @ u_prev + m_prev) ---
                utab = load_table(
                    utabp, "utab", u0 if ll == 1 else U_full[ll - 1], nu
                )
                m_newT = actp.tile([e, ms], F32, tag="mt")
                agg_side(
                    mat, nu, ms, utab, mT,
                    wm_t[:, li, :], bm_t[:, li : li + 1], m_newT,
                )
                mnat = to_nat(m_newT, ms, "mnat")
                if ll < n_layers:
                    nat_to_dram(mnat, ms, m_sh[ll])
                    nc.gpsimd.collective_compute(
                        "AllGather", mybir.AluOpType.bypass, replica_groups=rg,
                        ins=[m_sh[ll][:]], outs=[M_full[ll][:]],
                    )
                else:
                    nat_to_dram(mnat, ms, m3_o)
                # weighted movie activations for scoring
                mwT = natp.tile([e, ms], F32, tag="mw")
                nc.vector.tensor_scalar_mul(mwT[:], m_newT[:], ow_t[:, ll : ll + 1])
                mwnat = to_nat(mwT, ms, "mwnat")
                nat_to_dram(mwnat, ms, mw_sh[ll])
                nc.gpsimd.collective_compute(
                    "AllGather", mybir.AluOpType.bypass, replica_groups=rg,
                    ins=[mw_sh[ll][:]], outs=[MW_full[ll][:]],
                )
                uT, mT = u_newT, m_newT

            # ---- scoring ----
            if score_inputs_only:
                gsrc_u = [u0] * (n_layers + 1)
                gsrc_m = [m0w] * (n_layers + 1)
            else:
                gsrc_u = [u0] + [U_full[ll] for ll in range(1, n_layers + 1)]
                gsrc_m = [m0w] + [MW_full[ll] for ll in range(1, n_layers + 1)]
            scores = constp.tile([128, bchunks], F32)
            if not do_score:
                nc.gpsimd.memset(scores[:], 0.0)
            for d in range(n_layers + 1) if do_score else []:
                gu = workp.tile([128, bchunks, e], F32, tag="gu")
                gm = workp.tile([128, bchunks, e], F32, tag="gm")
                nc.gpsimd.dma_gather(
                    out_ap=gu[:], in_ap=gsrc_u[d][:], idxs_ap=uidx_t[:],
                    num_idxs=bs, num_idxs_reg=bs, elem_size=e,
                )
                nc.gpsimd.dma_gather(
                    out_ap=gm[:], in_ap=gsrc_m[d][:], idxs_ap=midx_t[:],
                    num_idxs=bs, num_idxs_reg=bs, elem_size=e,
                )
                for c in range(bchunks):
                    prod = workp.tile([128, e], F32, tag="prod")
                    part = workp.tile([128, 1], F32, tag="part")
                    nc.vector.tensor_mul(prod[:], gu[:, c, :], gm[:, c, :])
                    nc.vector.tensor_reduce(
                        part[:], prod[:], axis=mybir.AxisListType.X,
                        op=mybir.AluOpType.add,
                    )
                    if d == 0:
                        nc.vector.tensor_add(scores[:, c : c + 1], part[:], ob_t[:])
                    else:
                        nc.vector.tensor_add(
                            scores[:, c : c + 1], scores[:, c : c + 1], part[:]
                        )
            nc.sync.dma_start(scores_o[:], scores[:])

    nc.compile()
    return nc


# ---------------------------------------------------------------------------
# host side
# ---------------------------------------------------------------------------

_CACHE = {}


def _get_nc(nu, nm, e, n_layers, bs, ncores, adj_dt):
    key = (nu, nm, e, n_layers, bs, ncores, adj_dt)
    if key not in _CACHE:
        _CACHE[key] = build_kernel(nu, nm, e, n_layers, bs, ncores, adj_dt)
    return _CACHE[key]


def _pack_idx(idx, ncols):
    """int array [n] -> [128, n/16] int16, 16-partition wrap, replicated x8."""
    n = idx.shape[0]
    assert n % 16 == 0
    w = idx.reshape(n // 16, 16).T.astype(np.int16)  # [16, n/16]
    out = np.zeros((128, ncols), np.int16)
    out[:, : w.shape[1]] = np.tile(w, (8, 1))
    return out


def prepare(user_adj, movie_adj, user_id, movie_id,
            user_emb, movie_emb, user_Ws, user_bs,
            movie_Ws, movie_bs, out_W, out_b):
    """Host-side sharding/prep. Returns (nc, in_maps, dims)."""
    user_adj = np.asarray(user_adj, np.float32)
    movie_adj = np.asarray(movie_adj, np.float32)
    user_emb = np.asarray(user_emb, np.float32)
    movie_emb = np.asarray(movie_emb, np.float32)
    user_Ws = np.asarray(user_Ws, np.float32)
    user_bs = np.asarray(user_bs, np.float32)
    movie_Ws = np.asarray(movie_Ws, np.float32)
    movie_bs = np.asarray(movie_bs, np.float32)
    out_W = np.asarray(out_W, np.float32)
    out_b = np.asarray(out_b, np.float32)
    uid = np.asarray(user_id)
    mid = np.asarray(movie_id)

    nu, nm = user_adj.shape
    e = user_emb.shape[1]
    n_layers = user_Ws.shape[0]
    b = uid.shape[0]
    ncores = NCORES
    us, ms, bs = nu // ncores, nm // ncores, b // ncores
    adj_np = ml_dtypes.bfloat16 if ADJ_DT == BF16 else np.float32

    nc = _get_nc(nu, nm, e, n_layers, bs, ncores, ADJ_DT)

    # shared (replicated) host tensors
    wuT = np.ascontiguousarray(user_Ws.transpose(0, 2, 1))
    wmT = np.ascontiguousarray(movie_Ws.transpose(0, 2, 1))
    buT = np.ascontiguousarray((2.0 * user_bs).T)
    bmT = np.ascontiguousarray((2.0 * movie_bs).T)
    ow = out_W.reshape(n_layers + 1, e)
    owT = np.ascontiguousarray(ow.T)
    m0w = movie_emb * ow[0][None, :]
    ob = np.full((128, 1), np.float32(out_b.reshape(-1)[0]), np.float32)
    uaT = user_adj.T  # [nm, nu] view
    maT = movie_adj.T  # [nu, nm] view

    in_maps = []
    for c in range(ncores):
        usl = slice(c * us, (c + 1) * us)
        msl = slice(c * ms, (c + 1) * ms)
        bsl = slice(c * bs, (c + 1) * bs)
        in_maps.append({
            "uat": np.ascontiguousarray(uaT[:, usl]).astype(adj_np, copy=False),
            "mat": np.ascontiguousarray(maT[:, msl]).astype(adj_np, copy=False),
            "u0": user_emb,
            "m0": movie_emb,
            "m0w": m0w.astype(np.float32),
            "u0T": np.ascontiguousarray(user_emb[usl].T),
            "m0T": np.ascontiguousarray(movie_emb[msl].T),
            "wuT": wuT, "wmT": wmT, "buT": buT, "bmT": bmT,
            "owT": owT, "ob": ob,
            "uidx": _pack_idx(np.asarray(uid[bsl], np.int64), max(1, bs // 16)),
            "midx": _pack_idx(np.asarray(mid[bsl], np.int64), max(1, bs // 16)),
        })
    return nc, in_maps, (ncores, bs)


def assemble(results, ncores, bs):
    scores = np.concatenate(
        [results[c]["scores"].T.reshape(-1)[:bs] for c in range(ncores)]
    )
    u3 = np.concatenate([results[c]["u3"] for c in range(ncores)], axis=0)
    m3 = np.concatenate([results[c]["m3"] for c in range(ncores)], axis=0)
    return scores, u3, m3


def kernel(*args, _run_kwargs=None, _return_raw=False, **kwargs):
    nc, in_maps, (ncores, bs) = prepare(*args, **kwargs)
    kw = dict(_run_kwargs or {})
    res = run_bass_kernel_spmd(nc, in_maps, list(range(ncores)), **kw)
    out = assemble(res.results, ncores, bs)
    if _return_raw:
        return out, res
    return out
